# revision 51
# baseline (speedup 1.0000x reference)
import sys

if "/opt/trn_rl_repo" not in sys.path:
    sys.path.insert(0, "/opt/trn_rl_repo")

import numpy as np

B, T, C = 2, 2048, 2048
H, H_KV = 16, 8
D = C // H  # 128
NCORES = 8
HL = H // NCORES  # 2 local query heads per core; 1 kv head per core

F32R_SCALE = 0.08838834764831845  # 1/sqrt(128)


def build_nc(b=B, t=T, c=C, mmdt="bf16"):
    """Build the per-core Bass program. Same program on all 8 cores; the
    sharding lives entirely in the input data each core receives."""
    import concourse.bass as bass  # noqa: F401
    import concourse.mybir as mybir
    import concourse.tile as tile
    from concourse import bacc

    f32 = mybir.dt.float32
    f32r = mybir.dt.float32r if mmdt == "f32r" else mybir.dt.bfloat16
    f8 = mybir.dt.float8e4
    DR = mybir.MatmulPerfMode.DoubleRow
    EXP = mybir.ActivationFunctionType.Exp
    COPY = mybir.ActivationFunctionType.Copy
    MULT = mybir.AluOpType.mult
    ADD = mybir.AluOpType.add

    ncb = c // 128  # contraction blocks for projections
    nt = t // 512  # 512-wide q tiles
    njb_per_t = 512 // 128  # 4 k-blocks per 512 q-tile

    nc = bacc.Bacc("TRN2", target_bir_lowering=False, debug=False)

    xT = nc.dram_tensor("xT", [b, c, t], f32r, kind="ExternalInput")
    wq = nc.dram_tensor("wq", [c, HL * D], f32r, kind="ExternalInput")
    wk = nc.dram_tensor("wk", [c, D], f32r, kind="ExternalInput")
    wv = nc.dram_tensor("wv", [c, D], f32r, kind="ExternalInput")
    wp = nc.dram_tensor("wp", [HL * D, c], f32r, kind="ExternalInput")
    cos2 = nc.dram_tensor("cos2", [128, t], f32r, kind="ExternalInput")
    sin2 = nc.dram_tensor("sin2", [128, t], f32r, kind="ExternalInput")
    maskf = nc.dram_tensor("maskf", [128, 512], f32r, kind="ExternalInput")
    selv = nc.dram_tensor("selv", [128, 64], f8, kind="ExternalInput")
    ident = nc.dram_tensor("ident", [128, 128], f32r, kind="ExternalInput")
    y = nc.dram_tensor("y", [b, t, c], f32r, kind="ExternalOutput")

    with tile.TileContext(nc) as tc:
        with (
            tc.tile_pool(name="wts", bufs=1) as wpool,
            tc.tile_pool(name="data", bufs=1) as dpool,
            tc.tile_pool(name="work", bufs=2) as wkp,
            tc.tile_pool(name="psum", bufs=1, space="PSUM") as pp,
        ):
            # ---- resident weights / tables: need-order emission, byte-
            # balanced across the sync and scalar DMA queues (head is
            # HBM-bound; both queues are FIFO so emission order = need order)
            nw = max(ncb // 4, 1)  # cb chunks per wk/wv DMA
            wq_sbs, wk_sbs, wv_sbs = [], [], []
            qb = {"sync": 0, "scalar": 0, "gpsimd": 0}

            def emit(dst_ap, src_ap, nbytes, q="scalar"):
                (nc.sync if q == "sync" else nc.scalar).dma_start(dst_ap, src_ap)
                qb[q] += nbytes

            def emit_x(bi_, cb, p4, name, q=None):
                # x rides the sync + gpsimd DMA queues ONLY: DMA descriptor
                # issues block later compute ops on the issuing engine when
                # the hw queue backs up, and scalar carries the attention
                # exps. [128,1024] i4-pair tiles: 2KB rows halve the per-row
                # descriptor overhead vs 512-wide tiles.
                q = q or "sync"
                tag, bufs_ = ("xt", 26) if q == "sync" else ("xto", 8)
                t_ = wkp.tile([128, 1024], f32r, tag=tag, bufs=bufs_, name=name)
                eng = {"sync": nc.sync, "scalar": nc.scalar, "gpsimd": nc.gpsimd}[q]
                if bi_ == 0 and p4 == 0 and cb < 3:
                    # first tiles as two half transfers: the i4-0 half lands
                    # ~0.7us earlier, pulling the first matmul forward
                    eng.dma_start(t_[:, 0:512],
                                  xT[bi_, cb * 128 : (cb + 1) * 128,
                                     0:512])
                    eng.dma_start(t_[:, 512:1024],
                                  xT[bi_, cb * 128 : (cb + 1) * 128,
                                     512:1024])
                else:
                    eng.dma_start(t_[:], xT[bi_, cb * 128 : (cb + 1) * 128,
                                            p4 * 1024 : (p4 + 1) * 1024])
                qb[q] += 1024 * 128 * 2
                return t_

            # head: per-cb wq chunks + wk/wv chunks just ahead of their x
            # tile (batch 0 tile 0 is consumed cb-major: all four
            # projections per arriving x tile)
            XTP0 = {}
            XTP0[(0, 0, 0)] = emit_x(0, 0, 0, "xts0", q="sync")
            for cb in range(ncb):
                wq_i = wpool.tile([128, HL * D], f32r, name=f"wq{cb}")
                emit(wq_i[:], wq[cb * 128 : (cb + 1) * 128, :], HL * D * 128 * 2,
                     q="sync" if cb == 0 else "scalar")
                wq_sbs.append(wq_i)
                if cb % nw == 0:
                    wi = cb // nw
                    cbs = slice(wi * nw * 128, (wi + 1) * nw * 128)
                    wk_i = wpool.tile([128, nw * D], f32r, name=f"wk{wi}")
                    emit(
                        wk_i[:].rearrange("p (cb d) -> p cb d", d=D),
                        wk[cbs, :].rearrange("(cb p) d -> p cb d", p=128),
                        nw * D * 128 * 2,
                    )
                    wk_sbs.append(wk_i)
                    wv_i = wpool.tile([128, nw * D], f32r, name=f"wv{wi}")
                    emit(
                        wv_i[:].rearrange("p (cb d) -> p cb d", d=D),
                        wv[cbs, :].rearrange("(cb p) d -> p cb d", p=128),
                        nw * D * 128 * 2,
                    )
                    wv_sbs.append(wv_i)
                if cb > 0:
                    XTP0[(0, 0, cb)] = emit_x(0, cb, 0, "xts0")
            cos_sb = wpool.tile([128, t], f32r)
            emit(cos_sb[:], cos2[:, :], t * 128 * 2)
            sin_sb = wpool.tile([128, t], f32r)
            emit(sin_sb[:], sin2[:, :], t * 128 * 2)
            mask_sb = wpool.tile([128, 512], f32r)
            emit(mask_sb[:], maskf[:, :], 512 * 128 * 2)
            sel_sb = wpool.tile([128, 64], f8)
            emit(sel_sb[:], selv[:, :], 64 * 128)
            id_sb = wpool.tile([128, 128], f32r)
            emit(id_sb[:], ident[:, :], 128 * 128 * 2)
            id32_sb = wpool.tile([128, 128], f32)
            nc.vector.tensor_copy(id32_sb[:], id_sb[:])
            wp_sb = wpool.tile([128, HL * c], f32r)  # [p, (f, cout)]

            def emit_wp():
                emit(
                    wp_sb[:].rearrange("p (f n) -> p f n", n=c),
                    wp.rearrange("(f p) n -> p f n", p=128),
                    HL * c * 128 * 2,
                )

            warm = wpool.tile([128, 1], f32)
            nc.scalar.activation(warm[:], cos_sb[:, 0:1], EXP, scale=1.0)

            # batch 0's second i4-pair immediately behind the head pairs;
            # batch 1's x is emitted later, chunked between batch 0's
            # attention tiles (gpsimd queue), so its transfers stream
            # during attention without blocking the gpsimd broadcasts
            for cb_ in range(ncb):
                XTP0[(0, 1, cb_)] = emit_x(
                    0, cb_, 1, "xtc", q="scalar" if cb_ >= 10 else "sync"
                )
            # batch 1 p4-0: sync-only, ring-paced — transfers stream during
            # batch 0's late projections/attention (no broadcast conflicts:
            # gpsimd carries no lazy x; no y conflicts: emitted before any y)
            for cb_ in range(ncb):
                XTP0[(1, 0, cb_)] = emit_x(1, cb_, 0, "xtc", q="sync")
            xlazy = [(1, 1, cb_) for cb_ in range(ncb)]

            def emit_lazy_x(k):
                for _ in range(k):
                    if not xlazy:
                        return
                    bi_, p4_, cb_ = xlazy.pop(0)
                    XTP0[(bi_, p4_, cb_)] = emit_x(
                        bi_, cb_, p4_, "xtc", q="sync"
                    )

            swap_mask = [i ^ 1 for i in range(32)]

            def rope(dest, src, ts_):
                # dest = src*cosI + swap_adjacent(src)*sinS (pair-interleaved
                # head layout: host permuted Wq/Wk cols so rotate-half pairs
                # are adjacent partitions)
                ra = wkp.tile([128, 512], f32, tag="ra", bufs=2)
                rb = wkp.tile([128, 512], f32, tag="rb", bufs=2)
                nc.vector.tensor_mul(ra[:], src, cos_sb[:, ts_])
                nc.vector.stream_shuffle(rb[:], src, swap_mask)
                nc.vector.tensor_mul(rb[:], rb[:], sin_sb[:, ts_])
                nc.vector.tensor_add(dest, ra[:], rb[:])

            pending_x = []  # cross-batch deferred out-projection units
            yrows = {}
            RECT = {}  # (bi, i4) -> transposed reciprocal [q-part, (itl h)]
            for bi in range(b):
                # ---- per-batch persistent tiles ----
                QT = [dpool.tile([128, t], f32r, tag=f"qt{h}", name=f"QT{h}") for h in range(HL)]
                KT = dpool.tile([128, t], f32r, tag="kt")
                VT = dpool.tile([128, t], f32r, tag="vtt")
                Vn = dpool.tile([128, t], f32r, tag="vn")  # V natural [k, (jb d)]
                AT = [dpool.tile([128, t], f32r, tag=f"at{h}", name=f"AT{h}") for h in range(HL)]

                # ---- x tiles: one DMA per (i4-pair, cb) — fatter transfers
                # amortize the per-partition descriptor cost on the sync queue
                def xt_slice(i4, cb, last):
                    key = (bi, i4 // 2, cb)
                    tile_ = XTP0.pop(key) if (last and i4 % 2 == 1) else XTP0[key]
                    lo = (i4 % 2) * 512
                    return tile_[:, lo : lo + 512]

                # ---- QKV projections, kind-major passes (+ fused RoPE) ----
                def emit_transposes(i4):
                    ts_ = slice(i4 * 512, (i4 + 1) * 512)
                    pt = pp.tile([128, 512], f32r, tag="mm", bufs=3)
                    for jj in range(4):
                        nc.tensor.transpose(
                            pt[:, jj * 128 : (jj + 1) * 128],
                            VT[:, i4 * 512 + jj * 128 : i4 * 512 + (jj + 1) * 128],
                            id_sb[:],
                        )
                    nc.vector.tensor_copy(Vn[:, ts_], pt[:])

                pend_tp = None
                start_i4 = 0
                if bi == 0:
                    # tiles 0+1 cb-major together: each arriving x pair feeds
                    # 8 matmuls (all four projections x both i4 halves) using
                    # every psum bank as an accumulator — the head stays
                    # PE-bound instead of DMA-paced
                    acc0 = {
                        "q0": pp.tile([128, 512], f32, tag="pav", bufs=2, name="aq0"),
                        "q1": pp.tile([128, 512], f32, tag="pav", bufs=2, name="aq1"),
                        "k": pp.tile([128, 512], f32, tag="po", bufs=2, name="ak"),
                        "v": pp.tile([128, 512], f32, tag="po", bufs=2, name="av"),
                    }
                    acc1 = {
                        "q0": pp.tile([128, 512], f32, tag="mm", bufs=3, name="bq0"),
                        "q1": pp.tile([128, 512], f32, tag="mm", bufs=3, name="bq1"),
                        "k": pp.tile([128, 512], f32, tag="mm", bufs=3, name="bk"),
                        "v": pp.tile([128, 512], f32, tag="den", bufs=1, name="bv"),
                    }
                    for cb in range(ncb):
                        xtr0 = xt_slice(0, cb, last=False)
                        xtr1 = xt_slice(1, cb, last=True)
                        st, sp = (cb == 0), (cb == ncb - 1)
                        wi, cbl = cb // nw, cb % nw
                        ws = {
                            "q0": wq_sbs[cb][:, 0:128],
                            "q1": wq_sbs[cb][:, 128:256],
                            "k": wk_sbs[wi][:, cbl * 128 : (cbl + 1) * 128],
                            "v": wv_sbs[wi][:, cbl * 128 : (cbl + 1) * 128],
                        }
                        for kind in ("q0", "q1", "k", "v"):
                            nc.tensor.matmul(acc0[kind][:], ws[kind], xtr0, start=st, stop=sp)
                        for kind in ("q0", "q1", "k", "v"):
                            nc.tensor.matmul(acc1[kind][:], ws[kind], xtr1, start=st, stop=sp)
                    ts0, ts1 = slice(0, 512), slice(512, 1024)
                    # i4-1 ropes first: they free the mm ring for i4-2's
                    # passes. i4-0's ropes are deferred into the i4-3
                    # iteration so i4-2's ropes aren't queued behind them on
                    # the in-order vector engine (they gate i4-3's passes via
                    # the mm ring); attention needs QT/KT i4-0 much later.
                    rope(QT[0][:, ts1], acc1["q0"][:], ts1)
                    rope(QT[1][:, ts1], acc1["q1"][:], ts1)
                    rope(KT[:, ts1], acc1["k"][:], ts1)
                    nc.scalar.copy(VT[:, ts1], acc1["v"][:])
                    nc.scalar.copy(VT[:, ts0], acc0["v"][:])

                    def deferred_ropes():
                        rope(QT[0][:, ts0], acc0["q0"][:], ts0)
                        rope(QT[1][:, ts0], acc0["q1"][:], ts0)
                        rope(KT[:, ts0], acc0["k"][:], ts0)

                    emit_transposes(0)
                    pend_tp = 1
                    start_i4 = 2
                else:
                    deferred_ropes = None
                for i4 in range(start_i4, nt):
                    if bi == 0 and i4 == 3 and deferred_ropes is not None:
                        deferred_ropes()
                        deferred_ropes = None
                    if bi == 0 and i4 == 2:
                        emit_wp()  # 1MB off the head's HBM window
                    ts_ = slice(i4 * 512, (i4 + 1) * 512)
                    # four kind-major passes (q0, q1, k, v), each re-reading the
                    # resident x tiles, so each pass's RoPE/copy overlaps the
                    # next pass's matmuls.
                    def pass_(kind):
                        # v rides the den ring (idle during projections): the
                        # mm ring then never chains the next tile's passes
                        # through the v-pass release
                        tg, nbf = ("den", 1) if kind == "v" else ("mm", 3)
                        ps = pp.tile([128, 512], f32, tag=tg, bufs=nbf, name=f"ps{kind}")
                        for cb in range(ncb):
                            if cb % 4 == 3 and kind != "q0" and len(pending_x) > 6:
                                fn = pending_x.pop(0)
                                fn()
                            xtr = xt_slice(i4, cb, last=(kind == "v"))
                            st, sp = (cb == 0), (cb == ncb - 1)
                            wi, cbl = cb // nw, cb % nw
                            w_sb = {
                                "q0": lambda: wq_sbs[cb][:, 0:128],
                                "q1": lambda: wq_sbs[cb][:, 128:256],
                                "k": lambda: wk_sbs[wi][:, cbl * 128 : (cbl + 1) * 128],
                                "v": lambda: wv_sbs[wi][:, cbl * 128 : (cbl + 1) * 128],
                            }[kind]()
                            nc.tensor.matmul(ps[:], w_sb, xtr, start=st, stop=sp)
                        return ps

                    pq0 = pass_("q0")
                    if pend_tp is not None:
                        emit_transposes(pend_tp)
                    rope(QT[0][:, ts_], pq0[:], ts_)
                    pq1 = pass_("q1")
                    rope(QT[1][:, ts_], pq1[:], ts_)
                    pk = pass_("k")
                    rope(KT[:, ts_], pk[:], ts_)
                    pv = pass_("v")
                    nc.scalar.copy(VT[:, ts_], pv[:])
                    pend_tp = i4
                emit_transposes(pend_tp)

                # ---- attention (j-pipelined: QK of j runs while exp of j-1 is
                # consumed by den/AV) with interleaved out-projection units ----
                def emit_oproj_unit(it, n, ATl=None, bil=None, tail=False,
                                    split=False, ptags=("po", "po"), rot=0):
                    # one (row-block, col-slice) of the output projection; the
                    # po matmuls have no exp dependency, so they soak up PE
                    # bubbles in the attention j-loop. split=True: AT is raw
                    # (unnormalized) — per-head psum + rec folded into the
                    # copies as a per-partition scale (last-i4 units).
                    ATl = AT if ATl is None else ATl
                    bil = bi if bil is None else bil
                    if n == 0:
                        yrows[(bil, it)] = wkp.tile(
                            [128, c], f32r, tag="yout", bufs=4, name="yrow"
                        )
                    dst = yrows[(bil, it)][:, n * 512 : (n + 1) * 512]
                    if split:
                        recT = RECT[(bil, it // 4)]
                        itl = it % 4
                        nb = {"po": 2, "mm": 3}
                        po0 = pp.tile([128, 512], f32, tag=ptags[0],
                                      bufs=nb[ptags[0]], name="po0")
                        po1 = pp.tile([128, 512], f32, tag=ptags[1],
                                      bufs=nb[ptags[1]], name="po1")
                        nc.tensor.matmul(
                            po0[:], ATl[0][:, it * 128 : (it + 1) * 128],
                            wp_sb[:, n * 512 : (n + 1) * 512],
                            start=True, stop=True,
                        )
                        nc.tensor.matmul(
                            po1[:], ATl[1][:, it * 128 : (it + 1) * 128],
                            wp_sb[:, c + n * 512 : c + (n + 1) * 512],
                            start=True, stop=True,
                        )
                        sc = recT[:, 2 * itl : 2 * itl + 1]
                        sv = recT[:, 2 * itl + 1 : 2 * itl + 2]
                        nc.scalar.activation(dst, po0[:], COPY, scale=sc)
                        nc.vector.scalar_tensor_tensor(
                            dst, po1[:], sv, dst, op0=MULT, op1=ADD,
                        )
                    else:
                        nb = {"po": 2, "mm": 3, "pav": 2}
                        po = pp.tile([128, 512], f32, tag=ptags[0],
                                     bufs=nb[ptags[0]], name="po")
                        for hh in range(HL):
                            nc.tensor.matmul(
                                po[:],
                                ATl[hh][:, it * 128 : (it + 1) * 128],
                                wp_sb[:, hh * c + n * 512 : hh * c + (n + 1) * 512],
                                start=(hh == 0), stop=(hh == HL - 1),
                            )
                        if tail and n % 2 == 0:
                            nc.scalar.copy(dst, po[:])
                        else:
                            nc.vector.tensor_copy(dst, po[:])
                    if n == 3:
                        yr = yrows.pop((bil, it))
                        if tail or split:
                            nc.sync.dma_start(
                                y[bil, it * 128 : (it + 1) * 128, 0:1024],
                                yr[:, 0:1024],
                            )
                            nc.scalar.dma_start(
                                y[bil, it * 128 : (it + 1) * 128, 1024:2048],
                                yr[:, 1024:2048],
                            )
                        else:
                            nc.sync.dma_start(
                                y[bil, it * 128 : (it + 1) * 128, :], yr[:]
                            )

                def attn(i4, pending):
                    qs = slice(i4 * 512, (i4 + 1) * 512)
                    njb = njb_per_t * (i4 + 1)
                    # alternate the AV-accumulator ring per i4 so attn(i4+1)
                    # never waits on attn(i4)'s rec/normalize chain
                    ptag = ("pav", "po")[i4 % 2]
                    pav = [pp.tile([128, 512], f32, tag=ptag, bufs=2, name=f"pav{h}") for h in range(HL)]
                    pden2 = pp.tile([32, 512], f32, tag="den", bufs=1, name="pden2")
                    E2 = {}
                    offs = {}
                    for j in range(njb + 1):
                        if j < njb:
                            off = max(j - njb_per_t * i4, 0) * 128
                            offs[j] = off
                            # one fp8 E tile per j holding both heads: [p, (h q)]
                            E_ = wkp.tile([128, 2 * 512], f8, tag="E", bufs=10, name="E8")
                            for h in range(HL):
                                pst = pp.tile([128, 512], f32, tag="mm", bufs=3, name=f"pst{h}")
                                nc.tensor.matmul(
                                    pst[:, off:512],
                                    KT[:, j * 128 : (j + 1) * 128],
                                    QT[h][:, i4 * 512 + off : (i4 + 1) * 512],
                                    start=True, stop=True,
                                )
                                if j >= njb_per_t * i4:
                                    # additive causal mask (-1e8 below diag) pre-exp
                                    nc.vector.tensor_add(
                                        pst[:, off : off + 128],
                                        pst[:, off : off + 128],
                                        mask_sb[:, 384:512],
                                    )
                                nc.scalar.activation(
                                    E_[:, h * 512 + off : h * 512 + 512],
                                    pst[:, off:512], EXP,
                                    scale=F32R_SCALE,
                                )
                            E2[j] = E_
                        if j > 0:
                            jp = j - 1
                            o = offs[jp]
                            Ep = E2.pop(jp)
                            Epr = Ep[:].rearrange("p (kt q) -> p kt q", q=512)

                            def den_mm():
                                # DoubleRow selector: both heads' denominators
                                # in one fp8 matmul (h-slices are the 2 k-tiles)
                                nc.tensor.matmul(
                                    pden2[:, o:512],
                                    sel_sb[:].rearrange("p (kt m) -> p kt m", m=32),
                                    Epr[:, :, o:512],
                                    start=(jp == 0), stop=(jp == njb - 1),
                                    perf_mode=DR,
                                    skip_group_check=True,
                                )

                            def av_mm(h):
                                nc.tensor.matmul(
                                    pav[h][:, o:512],
                                    Vn[:, jp * 128 : (jp + 1) * 128],
                                    Ep[:, h * 512 + o : h * 512 + 512],
                                    start=(jp == 0), stop=(jp == njb - 1),
                                    skip_group_check=True,
                                )

                            den_mm()
                            iters_left = njb + 1 - j
                            k = min(
                                len(pending),
                                max(1, -(-len(pending) // max(iters_left, 1))),
                            )
                            for _ in range(k):
                                u = pending.pop(0)
                                if callable(u):
                                    u()
                                else:
                                    emit_oproj_unit(
                                        u[0], u[1], ptags=(u[2], u[2])
                                    )
                            av_mm(0)
                            av_mm(1)
                    rec2 = wkp.tile([2, 512], f32, tag="rec", bufs=2)
                    nc.vector.reciprocal_approx_fast(rec2[:], pden2[0:2, :])
                    if i4 == nt - 1:
                        # last i4: leave AT raw; transpose rec to [q-part, h]
                        # for the split units' per-partition copy scales —
                        # removes the broadcast+mul serial chain at the batch
                        # boundary / tail
                        dnt = pp.tile([128, 512], f32, tag="mm", bufs=3, name="dnt")
                        for itl in range(4):
                            nc.tensor.transpose(
                                dnt[:, itl * 2 : itl * 2 + 2],
                                rec2[:, itl * 128 : (itl + 1) * 128],
                                id32_sb[0:2, 0:2],
                            )
                        recT = wkp.tile([128, 8], f32, tag="recT", bufs=2)
                        nc.vector.tensor_copy(recT[:], dnt[:, 0:8])
                        RECT[(bi, i4)] = recT
                        nc.scalar.copy(AT[0][:, qs], pav[0][:])
                        nc.vector.tensor_copy(AT[1][:, qs], pav[1][:])
                    else:
                        rec1 = wkp.tile([1, 512], f32, tag="rec1", bufs=2)
                        nc.scalar.dma_start(rec1[:], rec2[1:2, :])
                        for h in range(HL):
                            rbc = wkp.tile([128, 512], f32, tag="rbc", bufs=2)
                            nc.gpsimd.partition_broadcast(
                                rbc[:], rec2[0:1, :] if h == 0 else rec1[:]
                            )
                            nc.vector.tensor_mul(AT[h][:, qs], pav[h][:], rbc[:])

                def oproj_units(i4):
                    utag = ("pav", "po")[i4 % 2]
                    return [
                        (it, n, utag)
                        for it in range(i4 * 4, (i4 + 1) * 4)
                        for n in range(4)
                    ]

                for i4 in range(nt):
                    attn(i4, oproj_units(i4 - 1) if i4 > 0 else pending_x)
                    if bi == 0 and i4 >= 2:
                        emit_lazy_x(8)  # gpsimd, after its last broadcast
                if bi < b - 1:
                    for it_, n_, _ut in oproj_units(nt - 1):
                        pending_x.append(
                            lambda it=it_, n=n_, ATl=AT, bil=bi: emit_oproj_unit(
                                it, n, ATl=ATl, bil=bil, split=True
                            )
                        )
                else:
                    # tail: split units, alternating psum rings (the attention
                    # rings are free now) for a deeper matmul/copy pipeline;
                    # copies rotate over scalar/vector/gpsimd
                    for ui, (it_, n_, _ut) in enumerate(oproj_units(nt - 1)):
                        ptags = ("mm", "mm") if ui % 2 == 0 else ("po", "po")
                        emit_oproj_unit(it_, n_, tail=True, split=True,
                                        ptags=ptags, rot=ui)

    nc.compile()
    return nc


def host_inputs(x, Wq, Wk, Wv, Wp, ncores=NCORES, mmdt="bf16"):
    import ml_dtypes

    mdt = np.float32 if mmdt == "f32r" else ml_dtypes.bfloat16
    """Per-core input dicts (sharding + layout prep on host)."""
    b, t, c = x.shape
    d = D
    xT = np.ascontiguousarray(np.transpose(x, (0, 2, 1)))  # [B, C, T]
    inv = (1.0 / (10000.0 ** (np.arange(0, d, 2, dtype=np.float32) / np.float32(d)))).astype(np.float32)
    pos = np.arange(t, dtype=np.float32)
    fr = np.outer(pos, inv).astype(np.float32)  # [T, 64]
    cosT = np.cos(fr).T.astype(np.float32)  # [64, T]
    sinT = np.sin(fr).T.astype(np.float32)
    # pair-interleaved rope tables: partition 2m,2m+1 <- freq m; sign -/+ on sin
    cosI = np.ascontiguousarray(np.repeat(cosT, 2, axis=0))  # [128, T]
    sinS = np.ascontiguousarray(
        np.stack([-sinT, sinT], axis=1).reshape(128, t)
    )
    # column permutation putting rope pair (m, m+64) at (2m, 2m+1), per head
    perm = np.stack([np.arange(64), np.arange(64) + 64], 1).reshape(128)
    triu = np.triu(np.ones((128, 128), np.float32))
    # additive causal mask: 0 where key<=query, -1e8 below the diagonal
    maskadd = np.where(triu > 0, np.float32(0.0), np.float32(-1e8))
    maskf = np.ascontiguousarray(
        np.concatenate([np.zeros((128, 384), np.float32), maskadd], 1)
    )
    # DoubleRow den selector: k-subtile h -> output partition h
    # lhsT [128, 2, 32]: subtile 0 col 0 = 1 (h0), subtile 1 col 1 = 1 (h1)
    selv = np.zeros((128, 64), np.float32)
    selv[:, 0] = 1.0
    selv[:, 32 + 1] = 1.0
    ident = np.eye(128, dtype=np.float32)

    def permute_heads(w):
        # w: [c, nheads*d] -> same with each head's columns permuted by perm
        nh = w.shape[1] // d
        wv_ = w.reshape(w.shape[0], nh, d)
        return np.ascontiguousarray(wv_[:, :, perm].reshape(w.shape))

    Wq_p = permute_heads(Wq)
    Wk_p = permute_heads(Wk)

    xTm = xT.astype(mdt) if mdt is not np.float32 else xT
    in_maps = []
    for ci in range(ncores):
        qs = slice(ci * HL * d, (ci + 1) * HL * d)
        in_maps.append(
            {
                "xT": xTm,
                "wq": np.ascontiguousarray(Wq_p[:, qs]).astype(mdt),
                "wk": np.ascontiguousarray(Wk_p[:, ci * d : (ci + 1) * d]).astype(mdt),
                "wv": np.ascontiguousarray(Wv[:, ci * d : (ci + 1) * d]).astype(mdt),
                "wp": np.ascontiguousarray(Wp[qs, :]).astype(mdt),
                "cos2": cosI.astype(mdt),
                "sin2": sinS.astype(mdt),
                "maskf": maskf.astype(mdt),
                "selv": selv.astype(ml_dtypes.float8_e4m3),
                "ident": ident.astype(mdt),
            }
        )
    return in_maps


_NC_CACHE = {}

MMDT = "bf16"


def _get_nc(mmdt=None):
    mmdt = mmdt or MMDT
    key = (B, T, C, mmdt)
    if key not in _NC_CACHE:
        _NC_CACHE[key] = build_nc(B, T, C, mmdt=mmdt)
    return _NC_CACHE[key]


def _install_cc_error_surfacing():
    """Make neuronx_cc hook failures print a real traceback instead of the
    opaque PJRT 'py_result' error."""
    try:
        from concourse import bass2jax

        bass2jax.install_neuronx_cc_hook()
        import libneuronxla

        if getattr(libneuronxla, "_tb_wrapped", False):
            return
        inner = libneuronxla.neuronx_cc

        def wrapped(*a, **k):
            try:
                return inner(*a, **k)
            except BaseException:
                import traceback

                traceback.print_exc()
                raise

        libneuronxla.neuronx_cc = wrapped
        libneuronxla._tb_wrapped = True
    except Exception:
        pass


def run_spmd(x, Wq, Wk, Wv, Wp, trace=False, mmdt=None):
    from concourse.bass_utils import run_bass_kernel_spmd

    mmdt = mmdt or MMDT
    _install_cc_error_surfacing()

    nc = _get_nc(mmdt)
    in_maps = host_inputs(x, Wq, Wk, Wv, Wp, mmdt=mmdt)
    last_err = None
    for attempt in range(3):
        try:
            res = run_bass_kernel_spmd(
                nc, in_maps, core_ids=list(range(NCORES)), trace=trace
            )
            break
        except Exception as e:  # transient NRT device faults: retry
            last_err = e
            import time as _time

            _time.sleep(5.0)
    else:
        raise last_err
    acc = res.results[0]["y"].astype(np.float64)
    for i in range(1, NCORES):
        acc += res.results[i]["y"]
    return acc.astype(np.float32), res


def kernel(x, Wq, Wk, Wv, Wp):
    out, _ = run_spmd(x, Wq, Wk, Wv, Wp, trace=False)
    return out



# revision 52
# speedup vs baseline: 1.1699x; 1.1699x over previous
import sys

if "/opt/trn_rl_repo" not in sys.path:
    sys.path.insert(0, "/opt/trn_rl_repo")

import numpy as np

B, T, C = 2, 2048, 2048
H, H_KV = 16, 8
D = C // H  # 128
NCORES = 8
HL = H // NCORES  # 2 local query heads per core; 1 kv head per core

F32R_SCALE = 0.08838834764831845  # 1/sqrt(128)


def build_nc(b=B, t=T, c=C, mmdt="bf16"):
    """Build the per-core Bass program. Same program on all 8 cores; the
    sharding lives entirely in the input data each core receives."""
    import concourse.bass as bass  # noqa: F401
    import concourse.mybir as mybir
    import concourse.tile as tile
    from concourse import bacc

    f32 = mybir.dt.float32
    f32r = mybir.dt.float32r if mmdt == "f32r" else mybir.dt.bfloat16
    f8 = mybir.dt.float8e4
    DR = mybir.MatmulPerfMode.DoubleRow
    EXP = mybir.ActivationFunctionType.Exp
    COPY = mybir.ActivationFunctionType.Copy
    MULT = mybir.AluOpType.mult
    ADD = mybir.AluOpType.add

    ncb = c // 128  # contraction blocks for projections
    nt = t // 512  # 512-wide q tiles
    njb_per_t = 512 // 128  # 4 k-blocks per 512 q-tile

    nc = bacc.Bacc("TRN2", target_bir_lowering=False, debug=False)

    xT = nc.dram_tensor("xT", [b, c, t], f32r, kind="ExternalInput")
    wq = nc.dram_tensor("wq", [c, HL * D], f32r, kind="ExternalInput")
    wk = nc.dram_tensor("wk", [c, D], f32r, kind="ExternalInput")
    wv = nc.dram_tensor("wv", [c, D], f32r, kind="ExternalInput")
    wp = nc.dram_tensor("wp", [HL * D, c], f32r, kind="ExternalInput")
    cos2 = nc.dram_tensor("cos2", [128, t], f32r, kind="ExternalInput")
    sin2 = nc.dram_tensor("sin2", [128, t], f32r, kind="ExternalInput")
    maskf = nc.dram_tensor("maskf", [128, 512], f32r, kind="ExternalInput")
    selv = nc.dram_tensor("selv", [128, 64], f8, kind="ExternalInput")
    ident = nc.dram_tensor("ident", [128, 128], f32r, kind="ExternalInput")
    y = nc.dram_tensor("y", [b, t, c], f32r, kind="ExternalOutput")

    with tile.TileContext(nc) as tc:
        with (
            tc.tile_pool(name="wts", bufs=1) as wpool,
            tc.tile_pool(name="data", bufs=1) as dpool,
            tc.tile_pool(name="work", bufs=2) as wkp,
            tc.tile_pool(name="psum", bufs=1, space="PSUM") as pp,
        ):
            # ---- resident weights / tables: need-order emission, byte-
            # balanced across the sync and scalar DMA queues (head is
            # HBM-bound; both queues are FIFO so emission order = need order)
            nw = max(ncb // 4, 1)  # cb chunks per wk/wv DMA
            wq_sbs, wk_sbs, wv_sbs = [], [], []
            qb = {"sync": 0, "scalar": 0, "gpsimd": 0}

            def emit(dst_ap, src_ap, nbytes, q="scalar"):
                (nc.sync if q == "sync" else nc.scalar).dma_start(dst_ap, src_ap)
                qb[q] += nbytes

            def emit_x(bi_, cb, p4, name, q=None):
                # x rides the sync + gpsimd DMA queues ONLY: DMA descriptor
                # issues block later compute ops on the issuing engine when
                # the hw queue backs up, and scalar carries the attention
                # exps. [128,1024] i4-pair tiles: 2KB rows halve the per-row
                # descriptor overhead vs 512-wide tiles.
                q = q or "sync"
                tag, bufs_ = ("xt", 26) if q == "sync" else ("xto", 8)
                t_ = wkp.tile([128, 1024], f32r, tag=tag, bufs=bufs_, name=name)
                eng = {"sync": nc.sync, "scalar": nc.scalar, "gpsimd": nc.gpsimd}[q]
                if bi_ == 0 and p4 == 0 and cb < 3:
                    # first tiles as two half transfers: the i4-0 half lands
                    # ~0.7us earlier, pulling the first matmul forward
                    eng.dma_start(t_[:, 0:512],
                                  xT[bi_, cb * 128 : (cb + 1) * 128,
                                     0:512])
                    eng.dma_start(t_[:, 512:1024],
                                  xT[bi_, cb * 128 : (cb + 1) * 128,
                                     512:1024])
                else:
                    eng.dma_start(t_[:], xT[bi_, cb * 128 : (cb + 1) * 128,
                                            p4 * 1024 : (p4 + 1) * 1024])
                qb[q] += 1024 * 128 * 2
                return t_

            # head: per-cb wq chunks + wk/wv chunks just ahead of their x
            # tile (batch 0 tile 0 is consumed cb-major: all four
            # projections per arriving x tile)
            XTP0 = {}
            XTP0[(0, 0, 0)] = emit_x(0, 0, 0, "xts0", q="sync")
            for cb in range(ncb):
                wq_i = wpool.tile([128, HL * D], f32r, name=f"wq{cb}")
                emit(wq_i[:], wq[cb * 128 : (cb + 1) * 128, :], HL * D * 128 * 2,
                     q="sync" if cb == 0 else "scalar")
                wq_sbs.append(wq_i)
                if cb % nw == 0:
                    wi = cb // nw
                    cbs = slice(wi * nw * 128, (wi + 1) * nw * 128)
                    wk_i = wpool.tile([128, nw * D], f32r, name=f"wk{wi}")
                    emit(
                        wk_i[:].rearrange("p (cb d) -> p cb d", d=D),
                        wk[cbs, :].rearrange("(cb p) d -> p cb d", p=128),
                        nw * D * 128 * 2,
                    )
                    wk_sbs.append(wk_i)
                    wv_i = wpool.tile([128, nw * D], f32r, name=f"wv{wi}")
                    emit(
                        wv_i[:].rearrange("p (cb d) -> p cb d", d=D),
                        wv[cbs, :].rearrange("(cb p) d -> p cb d", p=128),
                        nw * D * 128 * 2,
                    )
                    wv_sbs.append(wv_i)
                if cb > 0:
                    XTP0[(0, 0, cb)] = emit_x(0, cb, 0, "xts0")
            cos_sb = wpool.tile([128, t], f32r)
            emit(cos_sb[:], cos2[:, :], t * 128 * 2)
            sin_sb = wpool.tile([128, t], f32r)
            emit(sin_sb[:], sin2[:, :], t * 128 * 2)
            mask_sb = wpool.tile([128, 512], f32r)
            emit(mask_sb[:], maskf[:, :], 512 * 128 * 2)
            sel_sb = wpool.tile([128, 64], f8)
            emit(sel_sb[:], selv[:, :], 64 * 128)
            id_sb = wpool.tile([128, 128], f32r)
            emit(id_sb[:], ident[:, :], 128 * 128 * 2)
            id32_sb = wpool.tile([128, 128], f32)
            nc.vector.tensor_copy(id32_sb[:], id_sb[:])
            wp_sb = wpool.tile([128, HL * c], f32r)  # [p, (f, cout)]

            def emit_wp():
                emit(
                    wp_sb[:].rearrange("p (f n) -> p f n", n=c),
                    wp.rearrange("(f p) n -> p f n", p=128),
                    HL * c * 128 * 2,
                )

            warm = wpool.tile([128, 1], f32)
            nc.scalar.activation(warm[:], cos_sb[:, 0:1], EXP, scale=1.0)

            # batch 0's second i4-pair immediately behind the head pairs;
            # batch 1's x is emitted later, chunked between batch 0's
            # attention tiles (gpsimd queue), so its transfers stream
            # during attention without blocking the gpsimd broadcasts
            for cb_ in range(ncb):
                XTP0[(0, 1, cb_)] = emit_x(
                    0, cb_, 1, "xtc", q="scalar" if cb_ >= 10 else "sync"
                )
            # batch 1 p4-0: sync-only, ring-paced — transfers stream during
            # batch 0's late projections/attention (no broadcast conflicts:
            # gpsimd carries no lazy x; no y conflicts: emitted before any y)
            for cb_ in range(ncb):
                XTP0[(1, 0, cb_)] = emit_x(1, cb_, 0, "xtc", q="sync")
            xlazy = [(1, 1, cb_) for cb_ in range(ncb)]

            def emit_lazy_x(k):
                for _ in range(k):
                    if not xlazy:
                        return
                    bi_, p4_, cb_ = xlazy.pop(0)
                    XTP0[(bi_, p4_, cb_)] = emit_x(
                        bi_, cb_, p4_, "xtc", q="sync"
                    )

            swap_mask = [i ^ 1 for i in range(32)]

            def rope(dest, src, ts_):
                # dest = src*cosI + swap_adjacent(src)*sinS (pair-interleaved
                # head layout: host permuted Wq/Wk cols so rotate-half pairs
                # are adjacent partitions)
                ra = wkp.tile([128, 512], f32, tag="ra", bufs=2)
                rb = wkp.tile([128, 512], f32, tag="rb", bufs=2)
                nc.vector.tensor_mul(ra[:], src, cos_sb[:, ts_])
                nc.vector.stream_shuffle(rb[:], src, swap_mask)
                nc.vector.tensor_mul(rb[:], rb[:], sin_sb[:, ts_])
                nc.vector.tensor_add(dest, ra[:], rb[:])

            pending_x = []  # cross-batch deferred out-projection units
            yrows = {}
            RECT = {}  # (bi, i4) -> transposed reciprocal [q-part, (itl h)]
            for bi in range(b):
                # ---- per-batch persistent tiles ----
                QT = [dpool.tile([128, t], f32r, tag=f"qt{h}", name=f"QT{h}") for h in range(HL)]
                KT = dpool.tile([128, t], f32r, tag="kt")
                VT = dpool.tile([128, t], f32r, tag="vtt")
                Vn = dpool.tile([128, t], f32r, tag="vn")  # V natural [k, (jb d)]
                AT = [dpool.tile([128, t], f32r, tag=f"at{h}", name=f"AT{h}") for h in range(HL)]

                # ---- x tiles: one DMA per (i4-pair, cb) — fatter transfers
                # amortize the per-partition descriptor cost on the sync queue
                def xt_slice(i4, cb, last):
                    key = (bi, i4 // 2, cb)
                    tile_ = XTP0.pop(key) if (last and i4 % 2 == 1) else XTP0[key]
                    lo = (i4 % 2) * 512
                    return tile_[:, lo : lo + 512]

                # ---- QKV projections, kind-major passes (+ fused RoPE) ----
                def emit_transposes(i4):
                    ts_ = slice(i4 * 512, (i4 + 1) * 512)
                    pt = pp.tile([128, 512], f32r, tag="pav", bufs=2)
                    for jj in range(4):
                        nc.tensor.transpose(
                            pt[:, jj * 128 : (jj + 1) * 128],
                            VT[:, i4 * 512 + jj * 128 : i4 * 512 + (jj + 1) * 128],
                            id_sb[:],
                        )
                    nc.vector.tensor_copy(Vn[:, ts_], pt[:])

                pend_tp = None
                start_i4 = 0
                if bi == 0:
                    # tiles 0+1 cb-major together: each arriving x pair feeds
                    # 8 matmuls (all four projections x both i4 halves) using
                    # every psum bank as an accumulator — the head stays
                    # PE-bound instead of DMA-paced
                    acc0 = {
                        "q0": pp.tile([128, 512], f32, tag="pav", bufs=2, name="aq0"),
                        "q1": pp.tile([128, 512], f32, tag="pav", bufs=2, name="aq1"),
                        "k": pp.tile([128, 512], f32, tag="po", bufs=2, name="ak"),
                        "v": pp.tile([128, 512], f32, tag="po", bufs=2, name="av"),
                    }
                    acc1 = {
                        "q0": pp.tile([128, 512], f32, tag="mm", bufs=3, name="bq0"),
                        "q1": pp.tile([128, 512], f32, tag="mm", bufs=3, name="bq1"),
                        "k": pp.tile([128, 512], f32, tag="mm", bufs=3, name="bk"),
                        "v": pp.tile([128, 512], f32, tag="den", bufs=1, name="bv"),
                    }
                    for cb in range(ncb):
                        xtr0 = xt_slice(0, cb, last=False)
                        xtr1 = xt_slice(1, cb, last=True)
                        st, sp = (cb == 0), (cb == ncb - 1)
                        wi, cbl = cb // nw, cb % nw
                        ws = {
                            "q0": wq_sbs[cb][:, 0:128],
                            "q1": wq_sbs[cb][:, 128:256],
                            "k": wk_sbs[wi][:, cbl * 128 : (cbl + 1) * 128],
                            "v": wv_sbs[wi][:, cbl * 128 : (cbl + 1) * 128],
                        }
                        for kind in ("q0", "q1", "k", "v"):
                            nc.tensor.matmul(acc0[kind][:], ws[kind], xtr0, start=st, stop=sp)
                        for kind in ("q0", "q1", "k", "v"):
                            nc.tensor.matmul(acc1[kind][:], ws[kind], xtr1, start=st, stop=sp)
                    ts0, ts1 = slice(0, 512), slice(512, 1024)
                    # i4-1 ropes first: they free the mm ring for i4-2's passes
                    rope(QT[0][:, ts1], acc1["q0"][:], ts1)
                    rope(QT[1][:, ts1], acc1["q1"][:], ts1)
                    rope(KT[:, ts1], acc1["k"][:], ts1)
                    nc.scalar.copy(VT[:, ts1], acc1["v"][:])
                    rope(QT[0][:, ts0], acc0["q0"][:], ts0)
                    rope(QT[1][:, ts0], acc0["q1"][:], ts0)
                    rope(KT[:, ts0], acc0["k"][:], ts0)
                    nc.scalar.copy(VT[:, ts0], acc0["v"][:])
                    emit_transposes(0)
                    pend_tp = 1
                    start_i4 = 2
                for i4 in range(start_i4, nt):
                    if bi == 0 and i4 == 2:
                        emit_wp()  # 1MB off the head's HBM window
                    ts_ = slice(i4 * 512, (i4 + 1) * 512)
                    # four kind-major passes (q0, q1, k, v), each re-reading the
                    # resident x tiles, so each pass's RoPE/copy overlaps the
                    # next pass's matmuls.
                    def pass_(kind):
                        ps = pp.tile([128, 512], f32, tag="mm", bufs=3, name=f"ps{kind}")
                        for cb in range(ncb):
                            if cb % 4 == 3 and kind != "q0" and len(pending_x) > 6:
                                fn = pending_x.pop(0)
                                fn()
                            xtr = xt_slice(i4, cb, last=(kind == "v"))
                            st, sp = (cb == 0), (cb == ncb - 1)
                            wi, cbl = cb // nw, cb % nw
                            w_sb = {
                                "q0": lambda: wq_sbs[cb][:, 0:128],
                                "q1": lambda: wq_sbs[cb][:, 128:256],
                                "k": lambda: wk_sbs[wi][:, cbl * 128 : (cbl + 1) * 128],
                                "v": lambda: wv_sbs[wi][:, cbl * 128 : (cbl + 1) * 128],
                            }[kind]()
                            nc.tensor.matmul(ps[:], w_sb, xtr, start=st, stop=sp)
                        return ps

                    pq0 = pass_("q0")
                    if pend_tp is not None:
                        emit_transposes(pend_tp)
                    rope(QT[0][:, ts_], pq0[:], ts_)
                    pq1 = pass_("q1")
                    rope(QT[1][:, ts_], pq1[:], ts_)
                    pk = pass_("k")
                    rope(KT[:, ts_], pk[:], ts_)
                    pv = pass_("v")
                    nc.scalar.copy(VT[:, ts_], pv[:])
                    pend_tp = i4
                emit_transposes(pend_tp)

                # ---- attention (j-pipelined: QK of j runs while exp of j-1 is
                # consumed by den/AV) with interleaved out-projection units ----
                def emit_oproj_unit(it, n, ATl=None, bil=None, tail=False,
                                    split=False, ptags=("po", "po"), rot=0):
                    # one (row-block, col-slice) of the output projection; the
                    # po matmuls have no exp dependency, so they soak up PE
                    # bubbles in the attention j-loop. split=True: AT is raw
                    # (unnormalized) — per-head psum + rec folded into the
                    # copies as a per-partition scale (last-i4 units).
                    ATl = AT if ATl is None else ATl
                    bil = bi if bil is None else bil
                    if n == 0:
                        yrows[(bil, it)] = wkp.tile(
                            [128, c], f32r, tag="yout", bufs=4, name="yrow"
                        )
                    dst = yrows[(bil, it)][:, n * 512 : (n + 1) * 512]
                    if split:
                        recT = RECT[(bil, it // 4)]
                        itl = it % 4
                        nb = {"po": 2, "mm": 3}
                        po0 = pp.tile([128, 512], f32, tag=ptags[0],
                                      bufs=nb[ptags[0]], name="po0")
                        po1 = pp.tile([128, 512], f32, tag=ptags[1],
                                      bufs=nb[ptags[1]], name="po1")
                        nc.tensor.matmul(
                            po0[:], ATl[0][:, it * 128 : (it + 1) * 128],
                            wp_sb[:, n * 512 : (n + 1) * 512],
                            start=True, stop=True,
                        )
                        nc.tensor.matmul(
                            po1[:], ATl[1][:, it * 128 : (it + 1) * 128],
                            wp_sb[:, c + n * 512 : c + (n + 1) * 512],
                            start=True, stop=True,
                        )
                        sc = recT[:, 2 * itl : 2 * itl + 1]
                        sv = recT[:, 2 * itl + 1 : 2 * itl + 2]
                        nc.scalar.activation(dst, po0[:], COPY, scale=sc)
                        nc.vector.scalar_tensor_tensor(
                            dst, po1[:], sv, dst, op0=MULT, op1=ADD,
                        )
                    else:
                        nb = {"po": 2, "mm": 3, "pav": 2}
                        po = pp.tile([128, 512], f32, tag=ptags[0],
                                     bufs=nb[ptags[0]], name="po")
                        for hh in range(HL):
                            nc.tensor.matmul(
                                po[:],
                                ATl[hh][:, it * 128 : (it + 1) * 128],
                                wp_sb[:, hh * c + n * 512 : hh * c + (n + 1) * 512],
                                start=(hh == 0), stop=(hh == HL - 1),
                            )
                        if tail and n % 2 == 0:
                            nc.scalar.copy(dst, po[:])
                        else:
                            nc.vector.tensor_copy(dst, po[:])
                    if n == 3:
                        yr = yrows.pop((bil, it))
                        if tail or split:
                            nc.sync.dma_start(
                                y[bil, it * 128 : (it + 1) * 128, 0:1024],
                                yr[:, 0:1024],
                            )
                            nc.scalar.dma_start(
                                y[bil, it * 128 : (it + 1) * 128, 1024:2048],
                                yr[:, 1024:2048],
                            )
                        else:
                            nc.sync.dma_start(
                                y[bil, it * 128 : (it + 1) * 128, :], yr[:]
                            )

                def attn(i4, pending):
                    qs = slice(i4 * 512, (i4 + 1) * 512)
                    njb = njb_per_t * (i4 + 1)
                    # alternate the AV-accumulator ring per i4 so attn(i4+1)
                    # never waits on attn(i4)'s rec/normalize chain
                    ptag = ("pav", "po")[i4 % 2]
                    pav = [pp.tile([128, 512], f32, tag=ptag, bufs=2, name=f"pav{h}") for h in range(HL)]
                    pden2 = pp.tile([32, 512], f32, tag="den", bufs=1, name="pden2")
                    E2 = {}
                    offs = {}
                    for j in range(njb + 1):
                        if j < njb:
                            off = max(j - njb_per_t * i4, 0) * 128
                            offs[j] = off
                            # one fp8 E tile per j holding both heads: [p, (h q)]
                            E_ = wkp.tile([128, 2 * 512], f8, tag="E", bufs=10, name="E8")
                            for h in range(HL):
                                pst = pp.tile([128, 512], f32, tag="mm", bufs=3, name=f"pst{h}")
                                nc.tensor.matmul(
                                    pst[:, off:512],
                                    KT[:, j * 128 : (j + 1) * 128],
                                    QT[h][:, i4 * 512 + off : (i4 + 1) * 512],
                                    start=True, stop=True,
                                )
                                if j >= njb_per_t * i4:
                                    # additive causal mask (-1e8 below diag) pre-exp
                                    nc.vector.tensor_add(
                                        pst[:, off : off + 128],
                                        pst[:, off : off + 128],
                                        mask_sb[:, 384:512],
                                    )
                                nc.scalar.activation(
                                    E_[:, h * 512 + off : h * 512 + 512],
                                    pst[:, off:512], EXP,
                                    scale=F32R_SCALE,
                                )
                            E2[j] = E_
                        if j > 0:
                            jp = j - 1
                            o = offs[jp]
                            Ep = E2.pop(jp)
                            Epr = Ep[:].rearrange("p (kt q) -> p kt q", q=512)

                            def den_mm():
                                # DoubleRow selector: both heads' denominators
                                # in one fp8 matmul (h-slices are the 2 k-tiles)
                                nc.tensor.matmul(
                                    pden2[:, o:512],
                                    sel_sb[:].rearrange("p (kt m) -> p kt m", m=32),
                                    Epr[:, :, o:512],
                                    start=(jp == 0), stop=(jp == njb - 1),
                                    perf_mode=DR,
                                    skip_group_check=True,
                                )

                            def av_mm(h):
                                nc.tensor.matmul(
                                    pav[h][:, o:512],
                                    Vn[:, jp * 128 : (jp + 1) * 128],
                                    Ep[:, h * 512 + o : h * 512 + 512],
                                    start=(jp == 0), stop=(jp == njb - 1),
                                    skip_group_check=True,
                                )

                            den_mm()
                            iters_left = njb + 1 - j
                            k = min(
                                len(pending),
                                max(1, -(-len(pending) // max(iters_left, 1))),
                            )
                            for _ in range(k):
                                u = pending.pop(0)
                                if callable(u):
                                    u()
                                else:
                                    emit_oproj_unit(
                                        u[0], u[1], ptags=(u[2], u[2])
                                    )
                            av_mm(0)
                            av_mm(1)
                    rec2 = wkp.tile([2, 512], f32, tag="rec", bufs=2)
                    nc.vector.reciprocal_approx_fast(rec2[:], pden2[0:2, :])
                    if i4 == nt - 1:
                        # last i4: leave AT raw; transpose rec to [q-part, h]
                        # for the split units' per-partition copy scales —
                        # removes the broadcast+mul serial chain at the batch
                        # boundary / tail
                        dnt = pp.tile([128, 512], f32, tag="mm", bufs=3, name="dnt")
                        for itl in range(4):
                            nc.tensor.transpose(
                                dnt[:, itl * 2 : itl * 2 + 2],
                                rec2[:, itl * 128 : (itl + 1) * 128],
                                id32_sb[0:2, 0:2],
                            )
                        recT = wkp.tile([128, 8], f32, tag="recT", bufs=2)
                        nc.vector.tensor_copy(recT[:], dnt[:, 0:8])
                        RECT[(bi, i4)] = recT
                        nc.scalar.copy(AT[0][:, qs], pav[0][:])
                        nc.vector.tensor_copy(AT[1][:, qs], pav[1][:])
                    else:
                        rec1 = wkp.tile([1, 512], f32, tag="rec1", bufs=2)
                        nc.scalar.dma_start(rec1[:], rec2[1:2, :])
                        for h in range(HL):
                            rbc = wkp.tile([128, 512], f32, tag="rbc", bufs=2)
                            nc.gpsimd.partition_broadcast(
                                rbc[:], rec2[0:1, :] if h == 0 else rec1[:]
                            )
                            nc.vector.tensor_mul(AT[h][:, qs], pav[h][:], rbc[:])

                def oproj_units(i4):
                    utag = ("pav", "po")[i4 % 2]
                    return [
                        (it, n, utag)
                        for it in range(i4 * 4, (i4 + 1) * 4)
                        for n in range(4)
                    ]

                for i4 in range(nt):
                    attn(i4, oproj_units(i4 - 1) if i4 > 0 else pending_x)
                    if bi == 0 and i4 >= 2:
                        emit_lazy_x(8)  # gpsimd, after its last broadcast
                if bi < b - 1:
                    for it_, n_, _ut in oproj_units(nt - 1):
                        pending_x.append(
                            lambda it=it_, n=n_, ATl=AT, bil=bi: emit_oproj_unit(
                                it, n, ATl=ATl, bil=bil, split=True
                            )
                        )
                else:
                    # tail: split units, alternating psum rings (the attention
                    # rings are free now) for a deeper matmul/copy pipeline;
                    # copies rotate over scalar/vector/gpsimd
                    for ui, (it_, n_, _ut) in enumerate(oproj_units(nt - 1)):
                        ptags = ("mm", "mm") if ui % 2 == 0 else ("po", "po")
                        emit_oproj_unit(it_, n_, tail=True, split=True,
                                        ptags=ptags, rot=ui)

    nc.compile()
    return nc


def host_inputs(x, Wq, Wk, Wv, Wp, ncores=NCORES, mmdt="bf16"):
    import ml_dtypes

    mdt = np.float32 if mmdt == "f32r" else ml_dtypes.bfloat16
    """Per-core input dicts (sharding + layout prep on host)."""
    b, t, c = x.shape
    d = D
    xT = np.ascontiguousarray(np.transpose(x, (0, 2, 1)))  # [B, C, T]
    inv = (1.0 / (10000.0 ** (np.arange(0, d, 2, dtype=np.float32) / np.float32(d)))).astype(np.float32)
    pos = np.arange(t, dtype=np.float32)
    fr = np.outer(pos, inv).astype(np.float32)  # [T, 64]
    cosT = np.cos(fr).T.astype(np.float32)  # [64, T]
    sinT = np.sin(fr).T.astype(np.float32)
    # pair-interleaved rope tables: partition 2m,2m+1 <- freq m; sign -/+ on sin
    cosI = np.ascontiguousarray(np.repeat(cosT, 2, axis=0))  # [128, T]
    sinS = np.ascontiguousarray(
        np.stack([-sinT, sinT], axis=1).reshape(128, t)
    )
    # column permutation putting rope pair (m, m+64) at (2m, 2m+1), per head
    perm = np.stack([np.arange(64), np.arange(64) + 64], 1).reshape(128)
    triu = np.triu(np.ones((128, 128), np.float32))
    # additive causal mask: 0 where key<=query, -1e8 below the diagonal
    maskadd = np.where(triu > 0, np.float32(0.0), np.float32(-1e8))
    maskf = np.ascontiguousarray(
        np.concatenate([np.zeros((128, 384), np.float32), maskadd], 1)
    )
    # DoubleRow den selector: k-subtile h -> output partition h
    # lhsT [128, 2, 32]: subtile 0 col 0 = 1 (h0), subtile 1 col 1 = 1 (h1)
    selv = np.zeros((128, 64), np.float32)
    selv[:, 0] = 1.0
    selv[:, 32 + 1] = 1.0
    ident = np.eye(128, dtype=np.float32)

    def permute_heads(w):
        # w: [c, nheads*d] -> same with each head's columns permuted by perm
        nh = w.shape[1] // d
        wv_ = w.reshape(w.shape[0], nh, d)
        return np.ascontiguousarray(wv_[:, :, perm].reshape(w.shape))

    Wq_p = permute_heads(Wq)
    Wk_p = permute_heads(Wk)

    xTm = xT.astype(mdt) if mdt is not np.float32 else xT
    in_maps = []
    for ci in range(ncores):
        qs = slice(ci * HL * d, (ci + 1) * HL * d)
        in_maps.append(
            {
                "xT": xTm,
                "wq": np.ascontiguousarray(Wq_p[:, qs]).astype(mdt),
                "wk": np.ascontiguousarray(Wk_p[:, ci * d : (ci + 1) * d]).astype(mdt),
                "wv": np.ascontiguousarray(Wv[:, ci * d : (ci + 1) * d]).astype(mdt),
                "wp": np.ascontiguousarray(Wp[qs, :]).astype(mdt),
                "cos2": cosI.astype(mdt),
                "sin2": sinS.astype(mdt),
                "maskf": maskf.astype(mdt),
                "selv": selv.astype(ml_dtypes.float8_e4m3),
                "ident": ident.astype(mdt),
            }
        )
    return in_maps


_NC_CACHE = {}

MMDT = "bf16"


def _get_nc(mmdt=None):
    mmdt = mmdt or MMDT
    key = (B, T, C, mmdt)
    if key not in _NC_CACHE:
        _NC_CACHE[key] = build_nc(B, T, C, mmdt=mmdt)
    return _NC_CACHE[key]


def _install_cc_error_surfacing():
    """Make neuronx_cc hook failures print a real traceback instead of the
    opaque PJRT 'py_result' error."""
    try:
        from concourse import bass2jax

        bass2jax.install_neuronx_cc_hook()
        import libneuronxla

        if getattr(libneuronxla, "_tb_wrapped", False):
            return
        inner = libneuronxla.neuronx_cc

        def wrapped(*a, **k):
            try:
                return inner(*a, **k)
            except BaseException:
                import traceback

                traceback.print_exc()
                raise

        libneuronxla.neuronx_cc = wrapped
        libneuronxla._tb_wrapped = True
    except Exception:
        pass


def run_spmd(x, Wq, Wk, Wv, Wp, trace=False, mmdt=None):
    from concourse.bass_utils import run_bass_kernel_spmd

    mmdt = mmdt or MMDT
    _install_cc_error_surfacing()

    nc = _get_nc(mmdt)
    in_maps = host_inputs(x, Wq, Wk, Wv, Wp, mmdt=mmdt)
    last_err = None
    for attempt in range(3):
        try:
            res = run_bass_kernel_spmd(
                nc, in_maps, core_ids=list(range(NCORES)), trace=trace
            )
            break
        except Exception as e:  # transient NRT device faults: retry
            last_err = e
            import time as _time

            _time.sleep(5.0)
    else:
        raise last_err
    acc = res.results[0]["y"].astype(np.float64)
    for i in range(1, NCORES):
        acc += res.results[i]["y"]
    return acc.astype(np.float32), res


def kernel(x, Wq, Wk, Wv, Wp):
    out, _ = run_spmd(x, Wq, Wk, Wv, Wp, trace=False)
    return out

